# revision 1
# baseline (speedup 1.0000x reference)
"""MultiHeadAttention (B=4, S=2048, D=1024, H=16, causal + key mask) on 8 trn2 cores.

Sharding: Megatron-style tensor parallel over heads. Each core owns 2 heads:
column slices of Wq/Wk/Wv (D x 128), the matching row slice of Wp (128 x D).
Each core computes a partial output y_c = attn_c @ Wp_c; host sums the 8
partials and adds bp.

Per-core kernel (all matmuls float32r: full PE rate at N=512, ~2e-4 rel err):
  - x^T [D, B*S] streamed in chunks; projections produce Q^T/K^T
    [128 = 2 heads x 64, B, S] directly (W slice as lhsT, x^T as rhs).
  - V via PE transpose into [s, hd] layout + a ones column so the PV matmul
    also accumulates the softmax denominator (row 64 of the PV psum).
  - Scores computed transposed: S^T[k, q] = K^T_slice.T @ Q^T_slice (K=64).
    Both heads' score blocks land in one [128,1024] 2-bank PSUM tile ->
    ONE additive causal mask (DVE) + ONE exp (ScalarE, key-mask as
    per-partition bias) per k-block. No max-subtraction (logits are O(1)).
  - PV accumulates attnT[hd, q]; psum copied to SBUF fast (frees the bank),
    reciprocal (DVE) -> partition-broadcast (DMA) -> normalize into a
    dedicated attnT buffer (reuses the x-stream pool's SBUF space).
  - Output projection is emitted two groups behind so the normalize chain
    never head-of-line blocks the in-order PE queue.
"""

import numpy as np

P = 128
B, S, D, H = 4, 2048, 1024, 16
HD = D // H  # 64
NCORES = 8
HPC = H // NCORES  # 2 heads per core
BS = B * S  # 8192
NB = S // P  # 16 k-blocks per batch
NG = S // 512  # 4 q-groups per batch

_CACHE = {}


def _build_nc():
    import concourse.mybir as mybir
    from concourse import bacc
    from concourse.tile import TileContext
    from concourse.masks import make_identity
    from contextlib import ExitStack

    f32 = mybir.dt.float32
    f32r = mybir.dt.float32r
    AF = mybir.ActivationFunctionType

    nc = bacc.Bacc("TRN2", target_bir_lowering=False, debug=False,
                   num_devices=NCORES)

    xT_d = nc.dram_tensor("xT", [D, BS], f32r, kind="ExternalInput").ap()
    wq_d = nc.dram_tensor("wq", [D, P], f32r, kind="ExternalInput").ap()
    wk_d = nc.dram_tensor("wk", [D, P], f32r, kind="ExternalInput").ap()
    wv_d = nc.dram_tensor("wv", [D, P], f32r, kind="ExternalInput").ap()
    bq_d = nc.dram_tensor("bq", [P, 1], f32, kind="ExternalInput").ap()
    bk_d = nc.dram_tensor("bk", [P, 1], f32, kind="ExternalInput").ap()
    bv_d = nc.dram_tensor("bv", [P, 1], f32, kind="ExternalInput").ap()
    wp_d = nc.dram_tensor("wp", [P, D], f32r, kind="ExternalInput").ap()
    mb_d = nc.dram_tensor("maskb", [P, B * NB], f32, kind="ExternalInput").ap()
    cm_d = nc.dram_tensor("cmask", [P, 4, 1024], f32,
                          kind="ExternalInput").ap()
    yp_d = nc.dram_tensor("yp", [BS, D], f32, kind="ExternalOutput").ap()

    xT_r = xT_d.rearrange("(o p) n -> p o n", p=P)  # [128, 8, 8192]
    KD = D // P  # 8 contraction chunks

    with TileContext(nc) as tc:
        with ExitStack() as ctx:
            consts = ctx.enter_context(tc.tile_pool(name="consts", bufs=1))
            big = ctx.enter_context(tc.tile_pool(name="big", bufs=1))
            ptpool = ctx.enter_context(tc.tile_pool(name="ptpool", bufs=3))
            npool = ctx.enter_context(tc.tile_pool(name="npool", bufs=2))
            ypool = ctx.enter_context(tc.tile_pool(name="ypool", bufs=3))
            psum = ctx.enter_context(
                tc.tile_pool(name="psum", bufs=2, space="PSUM"))
            sc2pool = ctx.enter_context(
                tc.tile_pool(name="sc2pool", bufs=2, space="PSUM"))
            pvpool = ctx.enter_context(
                tc.tile_pool(name="pvpool", bufs=2, space="PSUM"))

            # ---- constants ----
            wq_sb = consts.tile([P, KD, P], f32r, tag="wq")
            wk_sb = consts.tile([P, KD, P], f32r, tag="wk")
            wv_sb = consts.tile([P, KD, P], f32r, tag="wv")
            nc.sync.dma_start(wq_sb[:], wq_d.rearrange("(o p) m -> p o m", p=P))
            nc.sync.dma_start(wk_sb[:], wk_d.rearrange("(o p) m -> p o m", p=P))
            nc.sync.dma_start(wv_sb[:], wv_d.rearrange("(o p) m -> p o m", p=P))
            wp_sb = consts.tile([P, D], f32r, tag="wp")
            nc.sync.dma_start(wp_sb[:], wp_d)
            bq_sb = consts.tile([P, 1], f32, tag="bq")
            bk_sb = consts.tile([P, 1], f32, tag="bk")
            bv_sb = consts.tile([P, 1], f32, tag="bv")
            nc.sync.dma_start(bq_sb[:], bq_d)
            nc.sync.dma_start(bk_sb[:], bk_d)
            nc.sync.dma_start(bv_sb[:], bv_d)
            mb_sb = consts.tile([P, B * NB], f32, tag="mb")
            nc.sync.dma_start(mb_sb[:], mb_d)
            cm_sb = consts.tile([P, 4, 1024], f32, tag="cm")
            nc.sync.dma_start(cm_sb[:], cm_d)
            ident = consts.tile([P, P], f32, tag="ident")
            make_identity(nc, ident[:])

            # ---- persistent activations ----
            qt_sb = big.tile([P, B, S], f32r, tag="qt")  # Q^T
            kt_sb = big.tile([P, B, S], f32r, tag="kt")  # K^T
            # V in [s, hd] layout + ones col: [p=s%128, h, b, sblock, 65]
            v_sb = big.tile([P, HPC, B, NB, HD + 1], f32r, tag="v")
            nc.vector.memset(v_sb[:, :, :, :, HD].bitcast(f32), 1.0)

            # ---- phase 1: projections (x-stream pools scoped here) ----
            with tc.tile_pool(name="xpool", bufs=2) as xpool, \
                 tc.tile_pool(name="vtpool", bufs=2) as vtpool:
                for c in range(BS // 512):  # 16 chunks of 512 rows, b-major
                    b, sc = divmod(c, NG)
                    xt = xpool.tile([P, KD, 512], f32r, tag="xt")
                    nc.sync.dma_start(xt[:], xT_r[:, :, c * 512:(c + 1) * 512])
                    ssl = slice(sc * 512, (sc + 1) * 512)

                    for which in range(3):
                        w_sb = (wq_sb, wk_sb, wv_sb)[which]
                        ps = psum.tile([P, 512], f32, tag="ps")
                        for o in range(KD):
                            nc.tensor.matmul(
                                ps[:], lhsT=w_sb[:, o, :], rhs=xt[:, o, :],
                                start=(o == 0), stop=(o == KD - 1))
                        if which == 0:
                            nc.scalar.activation(qt_sb[:, b, ssl], ps[:],
                                                 AF.Identity, bias=bq_sb[:])
                        elif which == 1:
                            nc.scalar.activation(kt_sb[:, b, ssl], ps[:],
                                                 AF.Identity, bias=bk_sb[:])
                        else:
                            vt = vtpool.tile([P, 512], f32, tag="vt")
                            nc.scalar.activation(vt[:], ps[:], AF.Identity,
                                                 bias=bv_sb[:])
                            for t in range(4):
                                trp = psum.tile([P, 512], f32, tag="ps")
                                nc.tensor.transpose(
                                    trp[:, :P], vt[:, t * P:(t + 1) * P],
                                    ident[:])
                                sb_i = sc * 4 + t
                                nc.vector.tensor_copy(
                                    v_sb[:, 0, b, sb_i, 0:HD], trp[:, 0:HD])
                                nc.vector.tensor_copy(
                                    v_sb[:, 1, b, sb_i, 0:HD],
                                    trp[:, HD:2 * HD])

            # attnT buffer (reuses the closed x-stream pools' SBUF space)
            atpool = ctx.enter_context(tc.tile_pool(name="atpool", bufs=1))
            at_sb = atpool.tile([P, B, S], f32r, tag="at")

            # ---- phase 2: attention + output projection ----
            def outproj(b, g):
                for qc in range(4):
                    q0 = g * 512 + qc * P
                    r0 = b * S + q0
                    y_sb = ypool.tile([P, D], f32, tag="y",
                                      name=f"y_{b}_{g}_{qc}")
                    for half in range(2):
                        yp_ps = psum.tile([P, 512], f32, tag="ps",
                                          name=f"yps_{b}_{g}_{qc}_{half}")
                        nc.tensor.matmul(
                            yp_ps[:],
                            lhsT=at_sb[:, b, q0:q0 + P],
                            rhs=wp_sb[:, half * 512:(half + 1) * 512],
                            start=True, stop=True)
                        ysl = y_sb[:, half * 512:(half + 1) * 512]
                        nc.scalar.activation(ysl, yp_ps[:], AF.Copy)
                    nc.sync.dma_start(yp_d[r0:r0 + P, :], y_sb[:])

            pending = []
            for b in range(B):
                for g in range(NG):
                    gsl = slice(g * 512, (g + 1) * 512)
                    nkb = 4 * (g + 1)
                    pvs = [pvpool.tile([P, 512], f32, tag="pv",
                                       name=f"pv_{b}_{g}_{h}")
                           for h in range(HPC)]
                    for kb in range(nkb):
                        j = kb - 4 * g
                        col = b * NB + kb
                        # deep-diagonal blocks (j>=2): q < 128*j is fully
                        # masked; restrict to q in [256,512) (N=256 keeps
                        # full f32r rate; contiguous APs only)
                        qo = 256 if j >= 2 else 0
                        sc2 = sc2pool.tile([P, 1024], f32, tag="sc2",
                                           name=f"sc2_{b}_{g}_{kb}")
                        for h in range(HPC):
                            hsl = slice(h * HD, (h + 1) * HD)
                            nc.tensor.matmul(
                                sc2[:, h * 512 + qo:(h + 1) * 512],
                                lhsT=kt_sb[hsl, b, kb * P:(kb + 1) * P],
                                rhs=qt_sb[hsl, b,
                                          g * 512 + qo:(g + 1) * 512],
                                start=True, stop=True)
                        pt = ptpool.tile([P, 1024], f32r, tag="pt")
                        if qo == 0:
                            if j >= 0:  # diagonal: additive causal mask
                                nc.vector.tensor_add(sc2[:], sc2[:],
                                                     cm_sb[:, j, :])
                            nc.scalar.activation(pt[:], sc2[:], AF.Exp,
                                                 bias=mb_sb[:, col:col + 1])
                        else:
                            for h in range(HPC):
                                hs = slice(h * 512 + qo, (h + 1) * 512)
                                nc.vector.tensor_add(sc2[:, hs], sc2[:, hs],
                                                     cm_sb[:, j, hs])
                                nc.scalar.activation(
                                    pt[:, hs], sc2[:, hs], AF.Exp,
                                    bias=mb_sb[:, col:col + 1])
                        for h in range(HPC):
                            nc.tensor.matmul(
                                pvs[h][0:HD + 1, qo:512],
                                lhsT=v_sb[:, h, b, kb, :],
                                rhs=pt[:, h * 512 + qo:(h + 1) * 512],
                                start=(kb == 0), stop=(kb == nkb - 1))
                    if len(pending) >= 2:
                        outproj(*pending.pop(0))
                    pending.append((b, g))
                    for h in range(HPC):
                        # free the pv psum slot fast: copy [65,512] to SBUF
                        pvs_sb = npool.tile([P, 512], f32, tag="pvs")
                        nc.scalar.activation(pvs_sb[0:HD + 1, :],
                                             pvs[h][0:HD + 1, :], AF.Copy)
                        # 1/sum(exp) (row 64), broadcast to 64 partitions
                        rec = npool.tile([P, 512], f32, tag="rec")
                        nc.vector.reciprocal(
                            rec[HD:HD + 1, :], pvs_sb[HD:HD + 1, :])
                        sx = npool.tile([HD, 512], f32, tag="sx")
                        nc.sync.dma_start(
                            sx[:],
                            rec[HD:HD + 1, None, :]
                            .to_broadcast((1, HD, 512)))
                        if h == 0:
                            nc.vector.tensor_mul(
                                at_sb[0:HD, b, gsl], pvs_sb[0:HD, :], sx[:])
                        else:
                            tmp = npool.tile([HD, 512], f32r, tag="tmp")
                            nc.vector.tensor_mul(
                                tmp[:], pvs_sb[0:HD, :], sx[:])
                            nc.sync.dma_start(at_sb[HD:2 * HD, b, gsl],
                                              tmp[:])

            for pg in pending:
                outproj(*pg)

    nc.compile()
    return nc


def _get_nc():
    if "nc" not in _CACHE:
        _CACHE["nc"] = _build_nc()
    return _CACHE["nc"]


def make_in_maps(x, attention_mask, Wq, bq, Wk, bk, Wv, bv, Wp, bp):
    """Host-side sharding: build the 8 per-core device input maps."""
    x = np.asarray(x, dtype=np.float32)
    scale = np.float32(1.0 / np.sqrt(HD))
    xT = np.ascontiguousarray(x.reshape(BS, D).T)  # [D, BS]
    mb = (np.asarray(attention_mask).astype(np.float32) - 1.0) * np.float32(1e9)
    mb = np.ascontiguousarray(
        mb.reshape(B, NB, P).transpose(2, 0, 1).reshape(P, B * NB))
    # causal diag masks (additive): 0 where 128*j + p <= f, else -1e9;
    # duplicated for the two head halves of the [128,1024] scores tile.
    pp = np.arange(P)[:, None]
    ff = np.arange(512)[None, :]
    cm = np.stack(
        [np.where(P * j + pp <= ff, 0.0, -1e9).astype(np.float32)
         for j in range(4)], axis=1)  # [128, 4, 512]
    cm = np.ascontiguousarray(np.concatenate([cm, cm], axis=-1))

    Wq = np.asarray(Wq, np.float32) * scale
    bq = np.asarray(bq, np.float32) * scale
    Wk = np.asarray(Wk, np.float32)
    bk = np.asarray(bk, np.float32)
    Wv = np.asarray(Wv, np.float32)
    bv = np.asarray(bv, np.float32)
    Wp = np.asarray(Wp, np.float32)

    in_maps = []
    for c in range(NCORES):
        cs = slice(c * P, (c + 1) * P)
        in_maps.append({
            "xT": xT,
            "wq": np.ascontiguousarray(Wq[:, cs]),
            "wk": np.ascontiguousarray(Wk[:, cs]),
            "wv": np.ascontiguousarray(Wv[:, cs]),
            "bq": np.ascontiguousarray(bq[cs].reshape(P, 1)),
            "bk": np.ascontiguousarray(bk[cs].reshape(P, 1)),
            "bv": np.ascontiguousarray(bv[cs].reshape(P, 1)),
            "wp": np.ascontiguousarray(Wp[cs, :]),
            "maskb": mb,
            "cmask": cm,
        })
    return in_maps


def run(inputs, trace=False, tmpdir=None):
    """Compile (cached) + run on 8 cores. Returns (output, BassKernelResults)."""
    from concourse import bass_utils
    nc = _get_nc()
    in_maps = make_in_maps(**inputs)
    kwargs = {}
    if trace:
        kwargs = dict(trace=True, tmpdir=tmpdir)
    res = bass_utils.run_bass_kernel_spmd(
        nc, in_maps, core_ids=list(range(NCORES)), **kwargs)
    acc = np.zeros((BS, D), dtype=np.float64)
    for r in res.results:
        acc += r["yp"].astype(np.float64)
    out = (acc + np.asarray(inputs["bp"], np.float64)[None, :]).astype(
        np.float32)
    return out.reshape(B, S, D), res


def kernel(**inputs) -> np.ndarray:
    out, _ = run(inputs, trace=False)
    return out



# revision 7
# speedup vs baseline: 1.3057x; 1.3057x over previous
"""MultiHeadAttention (B=4, S=2048, D=1024, H=16, causal + key mask) on 8 trn2 cores.

Sharding: Megatron-style tensor parallel over heads. Each core owns 2 heads:
column slices of Wq/Wk/Wv (D x 128), the matching row slice of Wp (128 x D).
Each core computes a partial output y_c = attn_c @ Wp_c; host sums the 8
partials (bf16) and adds bp + bv@Wp (the V bias commutes through softmax).

All matmuls bf16 (full PE rate at any N; rel-err budget 2e-2 >> bf16 noise).
Engine assignment keeps ScalarE (the exp engine) free of copies:
  - PE:   Q^T/K^T proj, V proj direct to [s,hd] (no transposes), scores
          (2 heads row-tiled concurrent), PV (ones-column denominator),
          output projection.
  - ACT:  softmax exp only (+ small V psum copies).
  - DVE:  causal-mask adds (one [128,128] triangle reused for every
          diagonal step), Q/K bias copies, pvs copy, fast reciprocal,
          outproj psum->sbuf copies.
  - GP:   normalize multiplies (SBUF-only; GpSimd has no PSUM port).
Emission interleaves proj(b+1) chunks with attention(b) groups so the PE
never idles past the HAM window; off-diagonal (mask-free) blocks run first
in each group so group-boundary DVE bursts hide behind them.  Exact causal
trimming in 128-col steps (bf16 keeps full rate below N=256).
"""

import numpy as np
import ml_dtypes

P = 128
B, S, D, H = 4, 2048, 1024, 16
HD = D // H  # 64
NCORES = 8
HPC = H // NCORES  # 2 heads per core
BS = B * S  # 8192
NB = S // P  # 16 k-blocks per batch
NG = S // 512  # 4 q-groups per batch
KD = D // P  # 8 contraction chunks

_CACHE = {}


def _build_nc():
    import concourse.mybir as mybir
    from concourse import bacc
    from concourse.tile import TileContext
    from contextlib import ExitStack

    f32 = mybir.dt.float32
    bf16 = mybir.dt.bfloat16
    AF = mybir.ActivationFunctionType

    nc = bacc.Bacc("TRN2", target_bir_lowering=False, debug=False,
                   num_devices=NCORES)

    xT_d = nc.dram_tensor("xT", [D, BS], bf16, kind="ExternalInput").ap()
    wq_d = nc.dram_tensor("wq", [D, P], bf16, kind="ExternalInput").ap()
    wk_d = nc.dram_tensor("wk", [D, P], bf16, kind="ExternalInput").ap()
    wv_d = nc.dram_tensor("wv", [D, P], bf16, kind="ExternalInput").ap()
    bq_d = nc.dram_tensor("bq", [P, 1], f32, kind="ExternalInput").ap()
    bk_d = nc.dram_tensor("bk", [P, 1], f32, kind="ExternalInput").ap()
    wp_d = nc.dram_tensor("wp", [P, D], bf16, kind="ExternalInput").ap()
    mb_d = nc.dram_tensor("maskb", [P, B * NB], f32, kind="ExternalInput").ap()
    cm_d = nc.dram_tensor("cmask", [P, P], f32, kind="ExternalInput").ap()
    yp_d = nc.dram_tensor("yp", [BS, D], bf16, kind="ExternalOutput").ap()

    xT_r = xT_d.rearrange("(o p) n -> p o n", p=P)  # [128, 8, 8192]

    with TileContext(nc) as tc:
        with ExitStack() as ctx:
            consts = ctx.enter_context(tc.tile_pool(name="consts", bufs=1))
            big = ctx.enter_context(tc.tile_pool(name="big", bufs=1))
            xpool = ctx.enter_context(tc.tile_pool(name="xpool", bufs=2))
            ptpool = ctx.enter_context(tc.tile_pool(name="ptpool", bufs=3))
            npool = ctx.enter_context(tc.tile_pool(name="npool", bufs=2))
            ypool = ctx.enter_context(tc.tile_pool(name="ypool", bufs=4))
            pspool = ctx.enter_context(
                tc.tile_pool(name="pspool", bufs=2, space="PSUM"))
            sc2pool = ctx.enter_context(
                tc.tile_pool(name="sc2pool", bufs=2, space="PSUM"))
            pvpool = ctx.enter_context(
                tc.tile_pool(name="pvpool", bufs=1, space="PSUM"))

            # ---- constants ----
            wq_sb = consts.tile([P, KD, P], bf16, tag="wq")
            wk_sb = consts.tile([P, KD, P], bf16, tag="wk")
            wv_sb = consts.tile([P, KD, P], bf16, tag="wv")
            nc.sync.dma_start(wq_sb[:], wq_d.rearrange("(o p) m -> p o m", p=P))
            nc.sync.dma_start(wk_sb[:], wk_d.rearrange("(o p) m -> p o m", p=P))
            nc.sync.dma_start(wv_sb[:], wv_d.rearrange("(o p) m -> p o m", p=P))
            wp_sb = consts.tile([P, D], bf16, tag="wp")
            nc.sync.dma_start(wp_sb[:], wp_d)
            bq_sb = consts.tile([P, 1], f32, tag="bq")
            bk_sb = consts.tile([P, 1], f32, tag="bk")
            nc.sync.dma_start(bq_sb[:], bq_d)
            nc.sync.dma_start(bk_sb[:], bk_d)
            mb_sb = consts.tile([P, B * NB], f32, tag="mb")
            nc.sync.dma_start(mb_sb[:], mb_d)
            cm_sb = consts.tile([P, P], f32, tag="cm")
            nc.sync.dma_start(cm_sb[:], cm_d)

            # ---- persistent activations ----
            qt_sb = big.tile([P, B, S], bf16, tag="qt")  # Q^T [2h x 64, q]
            kt_sb = big.tile([P, B, S], bf16, tag="kt")  # K^T
            at_sb = big.tile([P, B, S], bf16, tag="at")  # normalized attn^T
            # V in [s, hd] layout + ones col: [s%128, h, b, kb, 65]
            v_sb = big.tile([P, HPC, B, NB, HD + 1], bf16, tag="v")
            nc.vector.memset(v_sb[:, 0, :, :, HD], 1.0)
            nc.vector.memset(v_sb[:, 1, :, :, HD], 1.0)

            pending = []

            def proj_chunk(b, c):
                cg = b * NG + c
                xt = xpool.tile([P, KD, 512], bf16, tag="xt")
                nc.sync.dma_start(xt[:], xT_r[:, :, cg * 512:(cg + 1) * 512])
                ssl = slice(c * 512, (c + 1) * 512)
                for w_sb, b_sb, dst in ((wq_sb, bq_sb, qt_sb),
                                        (wk_sb, bk_sb, kt_sb)):
                    ps = pspool.tile([P, 512], f32, tag="ps")
                    for o in range(KD):
                        nc.tensor.matmul(
                            ps[:], lhsT=w_sb[:, o, :], rhs=xt[:, o, :],
                            start=(o == 0), stop=(o == KD - 1))
                    nc.vector.tensor_scalar_add(dst[:, b, ssl], ps[:], b_sb[:])
                # V direct into [s, hd]: out[s128, 128] = x_chunk^T @ Wv
                psv = pspool.tile([P, 512], f32, tag="ps")
                for sub in range(4):
                    for o in range(KD):
                        nc.tensor.matmul(
                            psv[:, sub * P:(sub + 1) * P],
                            lhsT=xt[:, o, sub * P:(sub + 1) * P],
                            rhs=wv_sb[:, o, :],
                            start=(o == 0), stop=(o == KD - 1))
                for sub in range(4):
                    kb = c * 4 + sub
                    for h in range(HPC):
                        nc.vector.tensor_copy(
                            v_sb[:, h, b, kb, 0:HD],
                            psv[:, sub * P + h * HD:sub * P + (h + 1) * HD])

            def outproj(b, g):
                for qc in range(4):
                    q0 = g * 512 + qc * P
                    r0 = b * S + q0
                    y_sb = ypool.tile([P, D], bf16, tag="y",
                                      name=f"y_{b}_{g}_{qc}")
                    for half in range(2):
                        yp_ps = pspool.tile([P, 512], f32, tag="ps",
                                            name=f"yps_{b}_{g}_{qc}_{half}")
                        nc.tensor.matmul(
                            yp_ps[:],
                            lhsT=at_sb[:, b, q0:q0 + P],
                            rhs=wp_sb[:, half * 512:(half + 1) * 512],
                            start=True, stop=True)
                        nc.vector.tensor_copy(
                            y_sb[:, half * 512:(half + 1) * 512], yp_ps[:])
                    nc.sync.dma_start(yp_d[r0:r0 + P, :], y_sb[:])

            def attn_group(b, g):
                gsl = slice(g * 512, (g + 1) * 512)
                nkb = 4 * (g + 1)
                pv = pvpool.tile([P, HPC, 512], f32, tag="pv",
                                 name=f"pv_{b}_{g}")
                for kb in range(nkb):
                    j = kb - 4 * g
                    col = b * NB + kb
                    qo = P * j if j > 0 else 0
                    sc2 = sc2pool.tile([P, HPC, 512], f32, tag="sc2",
                                       name=f"sc2_{b}_{g}_{kb}")
                    for h in range(HPC):
                        hsl = slice(h * HD, (h + 1) * HD)
                        nc.tensor.matmul(
                            sc2[:, h, qo:512],
                            lhsT=kt_sb[hsl, b, kb * P:(kb + 1) * P],
                            rhs=qt_sb[hsl, b, g * 512 + qo:(g + 1) * 512],
                            start=True, stop=True)
                    if j >= 0:  # one 128-wide triangle at the causal boundary
                        for h in range(HPC):
                            nc.vector.tensor_add(
                                sc2[:, h, qo:qo + P], sc2[:, h, qo:qo + P],
                                cm_sb[:])
                    pt = ptpool.tile([P, HPC, 512], bf16, tag="pt")
                    nc.scalar.activation(pt[:, :, qo:512], sc2[:, :, qo:512],
                                         AF.Exp, bias=mb_sb[:, col:col + 1])
                    for h in range(HPC):
                        nc.tensor.matmul(
                            pv[0:HD + 1, h, qo:512],
                            lhsT=v_sb[:, h, b, kb, :],
                            rhs=pt[:, h, qo:512],
                            start=(kb == 0), stop=(kb == nkb - 1))
                if len(pending) >= 2:
                    outproj(*pending.pop(0))
                # normalize: copy psum out fast, 1/denom, broadcast, scale
                pvs = npool.tile([P, HPC, 512], f32, tag="pvs")
                nc.vector.tensor_copy(pvs[0:HD + 1, :, :], pv[0:HD + 1, :, :])
                # 1/denom = exp(-ln(denom)) — ScalarE; DVE reciprocal is
                # ~6.5ns/elem and the custom-DVE fast path doesn't work in
                # this deployment.
                rln = npool.tile([1, HPC, 512], f32, tag="rln")
                nc.scalar.activation(rln[:], pvs[HD:HD + 1, :, :], AF.Ln)
                rec = npool.tile([1, HPC, 512], f32, tag="rec")
                nc.scalar.activation(rec[:], rln[:], AF.Exp, scale=-1.0)
                sx = npool.tile([HD, HPC, 512], f32, tag="sx")
                nc.sync.dma_start(
                    sx[:],
                    rec[0:1, None, :, :].to_broadcast((1, HD, HPC, 512)))
                nc.gpsimd.tensor_mul(
                    at_sb[0:HD, b, gsl], pvs[0:HD, 0, :], sx[:, 0, :])
                tmp = npool.tile([HD, 512], bf16, tag="tmp")
                nc.gpsimd.tensor_mul(tmp[:], pvs[0:HD, 1, :], sx[:, 1, :])
                nc.sync.dma_start(at_sb[HD:2 * HD, b, gsl], tmp[:])
                pending.append((b, g))

            # ---- batch-pipelined emission ----
            for step in range(B + 1):
                for c in range(NG):
                    if step < B:
                        proj_chunk(step, c)
                    if step >= 1:
                        attn_group(step - 1, c)
            for pg in pending:
                outproj(*pg)

    nc.compile()
    return nc


def _get_nc():
    if "nc" not in _CACHE:
        _CACHE["nc"] = _build_nc()
    return _CACHE["nc"]


def make_in_maps(x, attention_mask, Wq, bq, Wk, bk, Wv, bv, Wp, bp):
    """Host-side sharding: build the 8 per-core device input maps."""
    bf = ml_dtypes.bfloat16
    x = np.asarray(x, dtype=np.float32)
    scale = np.float32(1.0 / np.sqrt(HD))
    xT = np.ascontiguousarray(x.reshape(BS, D).T).astype(bf)  # [D, BS]
    mb = (np.asarray(attention_mask).astype(np.float32) - 1.0) * np.float32(1e9)
    mb = np.ascontiguousarray(
        mb.reshape(B, NB, P).transpose(2, 0, 1).reshape(P, B * NB))
    # causal boundary triangle (additive): 0 where col >= row else -1e9;
    # reused for every diagonal 128-step.
    pp = np.arange(P)[:, None]
    cc = np.arange(P)[None, :]
    cm = np.where(cc >= pp, 0.0, -1e9).astype(np.float32)

    Wq = np.asarray(Wq, np.float32) * scale
    bq = np.asarray(bq, np.float32) * scale
    Wk = np.asarray(Wk, np.float32)
    bk = np.asarray(bk, np.float32)
    Wv = np.asarray(Wv, np.float32)
    Wp = np.asarray(Wp, np.float32)

    in_maps = []
    for c in range(NCORES):
        cs = slice(c * P, (c + 1) * P)
        in_maps.append({
            "xT": xT,
            "wq": np.ascontiguousarray(Wq[:, cs]).astype(bf),
            "wk": np.ascontiguousarray(Wk[:, cs]).astype(bf),
            "wv": np.ascontiguousarray(Wv[:, cs]).astype(bf),
            "bq": np.ascontiguousarray(bq[cs].reshape(P, 1)),
            "bk": np.ascontiguousarray(bk[cs].reshape(P, 1)),
            "wp": np.ascontiguousarray(Wp[cs, :]).astype(bf),
            "maskb": mb,
            "cmask": cm,
        })
    return in_maps


def run(inputs, trace=False, tmpdir=None):
    """Compile (cached) + run on 8 cores. Returns (output, BassKernelResults)."""
    from concourse import bass_utils
    nc = _get_nc()
    in_maps = make_in_maps(**inputs)
    kwargs = {}
    if trace:
        kwargs = dict(trace=True, tmpdir=tmpdir)
    res = bass_utils.run_bass_kernel_spmd(
        nc, in_maps, core_ids=list(range(NCORES)), **kwargs)
    acc = np.zeros((BS, D), dtype=np.float64)
    for r in res.results:
        acc += r["yp"].astype(np.float64)
    # V-bias commutes through softmax (weights sum to 1): + bv @ Wp
    bias = (np.asarray(inputs["bp"], np.float64)
            + np.asarray(inputs["bv"], np.float64)
            @ np.asarray(inputs["Wp"], np.float64))
    out = (acc + bias[None, :]).astype(np.float32)
    return out.reshape(B, S, D), res


def kernel(**inputs) -> np.ndarray:
    out, _ = run(inputs, trace=False)
    return out


# revision 14
# speedup vs baseline: 1.3711x; 1.0500x over previous
"""MultiHeadAttention (B=4, S=2048, D=1024, H=16, causal + key mask) on 8 trn2 cores.

Sharding: Megatron-style tensor parallel over heads. Each core owns 2 heads:
column slices of Wq/Wk/Wv (D x 128), the matching row slice of Wp (128 x D).
Each core computes a partial output y_c = attn_c @ Wp_c; host sums the 8
partials (bf16) and adds bp + bv@Wp (the V bias commutes through softmax).

All matmuls bf16 (full PE rate at any N; rel-err budget 2e-2 >> bf16 noise).
Engine assignment keeps ScalarE (the exp engine) free of copies:
  - PE:   Q^T/K^T/V^T projections (weights stationary, LDW hidden), V^T->V
          via PE transpose, scores (2 heads row-tiled concurrent), PV with a
          ones-column accumulating the softmax denominator, out projection.
  - ACT:  softmax exp; 1/denom = exp(-ln d) batched once per batch so the
          Ln<->Exp activation-table switch costs 2 loads/batch, not 2/group.
  - DVE:  causal-mask adds (one [128,128] triangle reused for every diagonal
          step), Q/K bias copies, V/pvs copies, outproj psum->sbuf casts.
  - GP:   normalize multiplies (SBUF-only; GpSimd has no PSUM port).
Emission interleaves proj(b+1) chunks with attention(b) groups so the PE
never idles past the HAM window; off-diagonal (mask-free) blocks run first
in each group so group-boundary DVE bursts hide behind them. Exact causal
trimming in 128-col steps (bf16 keeps full rate below N=256). Normalization
is deferred to the end of each batch; output projection lags one batch and
fills the PE between the next batch's attention groups.
"""

import numpy as np
import ml_dtypes

P = 128
B, S, D, H = 4, 2048, 1024, 16
HD = D // H  # 64
NCORES = 8
HPC = H // NCORES  # 2 heads per core
BS = B * S  # 8192
NB = S // P  # 16 k-blocks per batch
NG = S // 512  # 4 q-groups per batch
KD = D // P  # 8 contraction chunks

_CACHE = {}


def _build_nc():
    import concourse.mybir as mybir
    from concourse import bacc
    from concourse.tile import TileContext
    from concourse.masks import make_identity
    from contextlib import ExitStack

    f32 = mybir.dt.float32
    bf16 = mybir.dt.bfloat16
    AF = mybir.ActivationFunctionType

    nc = bacc.Bacc("TRN2", target_bir_lowering=False, debug=False,
                   num_devices=NCORES)

    xT_d = nc.dram_tensor("xT", [D, BS], bf16, kind="ExternalInput").ap()
    # weights pre-laid-out on host as [P, KD, P] / [P, D] for contiguous DMA
    wq_d = nc.dram_tensor("wq", [P, KD * P], bf16, kind="ExternalInput").ap()
    wk_d = nc.dram_tensor("wk", [P, KD * P], bf16, kind="ExternalInput").ap()
    wv_d = nc.dram_tensor("wv", [P, KD * P], bf16, kind="ExternalInput").ap()
    bq_d = nc.dram_tensor("bq", [P, 1], f32, kind="ExternalInput").ap()
    bk_d = nc.dram_tensor("bk", [P, 1], f32, kind="ExternalInput").ap()
    wp_d = nc.dram_tensor("wp", [P, D], bf16, kind="ExternalInput").ap()
    mb_d = nc.dram_tensor("maskb", [P, B * NB], f32, kind="ExternalInput").ap()
    cm_d = nc.dram_tensor("cmask", [P, P], f32, kind="ExternalInput").ap()
    yp_d = nc.dram_tensor("yp", [BS, D], bf16, kind="ExternalOutput").ap()

    xT_r = xT_d.rearrange("(o p) n -> p o n", p=P)  # [128, 8, 8192]

    with TileContext(nc) as tc:
        with ExitStack() as ctx:
            consts = ctx.enter_context(tc.tile_pool(name="consts", bufs=1))
            big = ctx.enter_context(tc.tile_pool(name="big", bufs=1))
            xpool = ctx.enter_context(tc.tile_pool(name="xpool", bufs=3))
            vtpool = ctx.enter_context(tc.tile_pool(name="vtpool", bufs=2))
            ptpool = ctx.enter_context(tc.tile_pool(name="ptpool", bufs=3))
            pvspool = ctx.enter_context(tc.tile_pool(name="pvspool", bufs=2))
            npool = ctx.enter_context(tc.tile_pool(name="npool", bufs=2))
            rpool = ctx.enter_context(tc.tile_pool(name="rpool", bufs=1))
            ypool = ctx.enter_context(tc.tile_pool(name="ypool", bufs=4))
            pspool = ctx.enter_context(
                tc.tile_pool(name="pspool", bufs=2, space="PSUM"))
            sc2pool = ctx.enter_context(
                tc.tile_pool(name="sc2pool", bufs=2, space="PSUM"))
            pvpool = ctx.enter_context(
                tc.tile_pool(name="pvpool", bufs=1, space="PSUM"))

            # ---- constants ----
            wq_sb = consts.tile([P, KD, P], bf16, tag="wq")
            wk_sb = consts.tile([P, KD, P], bf16, tag="wk")
            wv_sb = consts.tile([P, KD, P], bf16, tag="wv")
            nc.sync.dma_start(wq_sb[:], wq_d.rearrange("p (o m) -> p o m", m=P))
            nc.sync.dma_start(wk_sb[:], wk_d.rearrange("p (o m) -> p o m", m=P))
            nc.sync.dma_start(wv_sb[:], wv_d.rearrange("p (o m) -> p o m", m=P))
            wp_sb = consts.tile([P, D], bf16, tag="wp")
            nc.sync.dma_start(wp_sb[:], wp_d)
            bq_sb = consts.tile([P, 1], f32, tag="bq")
            bk_sb = consts.tile([P, 1], f32, tag="bk")
            nc.sync.dma_start(bq_sb[:], bq_d)
            nc.sync.dma_start(bk_sb[:], bk_d)
            mb_sb = consts.tile([P, B * NB], f32, tag="mb")
            nc.sync.dma_start(mb_sb[:], mb_d)
            cm_sb = consts.tile([P, P], f32, tag="cm")
            nc.sync.dma_start(cm_sb[:], cm_d)
            ident = consts.tile([P, P], f32, tag="ident")
            make_identity(nc, ident[:])

            # ---- persistent activations ----
            qt_sb = big.tile([P, B, S], bf16, tag="qt")  # Q^T [2h x 64, q]
            kt_sb = big.tile([P, B, S], bf16, tag="kt")  # K^T
            at_sb = big.tile([P, B, S], bf16, tag="at")  # normalized attn^T
            # V in [s, hd] layout + ones col: [s%128, b, kb, h, 65]
            v_sb = big.tile([P, B, NB, HPC, HD + 1], bf16, tag="v")
            nc.vector.memset(v_sb[:, :, :, :, HD], 1.0)

            pending = []

            def proj_chunk(b, c):
                cg = b * NG + c
                xt = xpool.tile([P, KD, 512], bf16, tag="xt")
                nc.sync.dma_start(xt[:], xT_r[:, :, cg * 512:(cg + 1) * 512])
                ssl = slice(c * 512, (c + 1) * 512)
                for w_sb, b_sb, dst in ((wq_sb, bq_sb, qt_sb),
                                        (wk_sb, bk_sb, kt_sb)):
                    ps = pspool.tile([P, 512], f32, tag="ps")
                    for o in range(KD):
                        nc.tensor.matmul(
                            ps[:], lhsT=w_sb[:, o, :], rhs=xt[:, o, :],
                            start=(o == 0), stop=(o == KD - 1))
                    nc.vector.tensor_scalar_add(dst[:, b, ssl], ps[:], b_sb[:])
                # V^T, then PE-transpose into [s, hd]
                psv = pspool.tile([P, 512], f32, tag="ps")
                for o in range(KD):
                    nc.tensor.matmul(
                        psv[:], lhsT=wv_sb[:, o, :], rhs=xt[:, o, :],
                        start=(o == 0), stop=(o == KD - 1))
                vt = vtpool.tile([P, 512], f32, tag="vt")
                nc.vector.tensor_copy(vt[:], psv[:])
                tps = pspool.tile([P, 4, HPC, HD], f32, tag="ps")
                for t in range(4):
                    nc.tensor.transpose(
                        tps[:, t, :, :], vt[:, t * P:(t + 1) * P], ident[:])
                nc.vector.tensor_copy(
                    v_sb[:, b, c * 4:c * 4 + 4, :, 0:HD], tps[:])

            def outproj(b, g):
                for qc in range(4):
                    q0 = g * 512 + qc * P
                    r0 = b * S + q0
                    y_sb = ypool.tile([P, D], bf16, tag="y",
                                      name=f"y_{b}_{g}_{qc}")
                    for half in range(2):
                        yp_ps = pspool.tile([P, 512], f32, tag="ps",
                                            name=f"yps_{b}_{g}_{qc}_{half}")
                        nc.tensor.matmul(
                            yp_ps[:],
                            lhsT=at_sb[:, b, q0:q0 + P],
                            rhs=wp_sb[:, half * 512:(half + 1) * 512],
                            start=True, stop=True)
                        nc.vector.tensor_copy(
                            y_sb[:, half * 512:(half + 1) * 512], yp_ps[:])
                    nc.sync.dma_start(yp_d[r0:r0 + P, :], y_sb[:])

            def attn_group(b, g, pvs_b):
                nkb = 4 * (g + 1)
                pv = pvpool.tile([P, HPC, 512], f32, tag="pv",
                                 name=f"pv_{b}_{g}")
                for kb in range(nkb):
                    j = kb - 4 * g
                    col = b * NB + kb
                    qo = P * j if j > 0 else 0
                    sc2 = sc2pool.tile([P, HPC, 512], f32, tag="sc2",
                                       name=f"sc2_{b}_{g}_{kb}")
                    for h in range(HPC):
                        hsl = slice(h * HD, (h + 1) * HD)
                        nc.tensor.matmul(
                            sc2[:, h, qo:512],
                            lhsT=kt_sb[hsl, b, kb * P:(kb + 1) * P],
                            rhs=qt_sb[hsl, b, g * 512 + qo:(g + 1) * 512],
                            start=True, stop=True)
                    if j >= 0:  # one 128-wide triangle at the causal boundary
                        for h in range(HPC):
                            nc.vector.tensor_add(
                                sc2[:, h, qo:qo + P], sc2[:, h, qo:qo + P],
                                cm_sb[:])
                    pt = ptpool.tile([P, HPC, 512], bf16, tag="pt")
                    nc.scalar.activation(pt[:, :, qo:512], sc2[:, :, qo:512],
                                         AF.Exp, bias=mb_sb[:, col:col + 1])
                    for h in range(HPC):
                        nc.tensor.matmul(
                            pv[0:HD + 1, h, qo:512],
                            lhsT=v_sb[:, b, kb, h, :],
                            rhs=pt[:, h, qo:512],
                            start=(kb == 0), stop=(kb == nkb - 1))
                if pending:
                    outproj(*pending.pop(0))
                # drain psum (unnormalized attn + denominators) to SBUF
                nc.vector.tensor_copy(pvs_b[0:HD + 1, g, :, :],
                                      pv[0:HD + 1, :, :])

            def normalize_batch(b, pvs_b):
                # 1/denom = exp(-ln d): Ln x4 then Exp x4 so the act-table
                # switches only twice per batch.
                rln = rpool.tile([1, NG, HPC, 512], f32, tag="rln")
                rec = rpool.tile([1, NG, HPC, 512], f32, tag="rec")
                for g in range(NG):
                    nc.scalar.activation(rln[:, g, :, :],
                                         pvs_b[HD:HD + 1, g, :, :], AF.Ln)
                for g in range(NG):
                    nc.scalar.activation(rec[:, g, :, :], rln[:, g, :, :],
                                         AF.Exp, scale=-1.0)
                for g in range(NG):
                    gsl = slice(g * 512, (g + 1) * 512)
                    sx = npool.tile([HD, HPC, 512], f32, tag="sx",
                                    name=f"sx_{b}_{g}")
                    nc.sync.dma_start(
                        sx[:], rec[0:1, None, g, :, :]
                        .to_broadcast((1, HD, HPC, 512)))
                    nc.gpsimd.tensor_mul(
                        at_sb[0:HD, b, gsl], pvs_b[0:HD, g, 0, :], sx[:, 0, :])
                    tmp = npool.tile([HD, 512], bf16, tag="tmp",
                                     name=f"tmp_{b}_{g}")
                    nc.gpsimd.tensor_mul(tmp[:], pvs_b[0:HD, g, 1, :],
                                         sx[:, 1, :])
                    nc.sync.dma_start(at_sb[HD:2 * HD, b, gsl], tmp[:])
                    pending.append((b, g))

            # ---- batch-pipelined emission ----
            pvs_tiles = {}
            for step in range(B + 1):
                if step < B:
                    pvs_tiles[step] = pvspool.tile(
                        [P, NG, HPC, 512], f32, tag="pvs", name=f"pvs_{step}")
                for c in range(NG):
                    if step < B:
                        proj_chunk(step, c)
                    if step >= 1:
                        attn_group(step - 1, c, pvs_tiles[step - 1])
                if step >= 1:
                    normalize_batch(step - 1, pvs_tiles[step - 1])
            for pg in pending:
                outproj(*pg)

    nc.compile()
    return nc


def _get_nc():
    if "nc" not in _CACHE:
        _CACHE["nc"] = _build_nc()
    return _CACHE["nc"]


def make_in_maps(x, attention_mask, Wq, bq, Wk, bk, Wv, bv, Wp, bp):
    """Host-side sharding: build the 8 per-core device input maps."""
    bf = ml_dtypes.bfloat16
    x = np.asarray(x, dtype=np.float32)
    scale = np.float32(1.0 / np.sqrt(HD))
    xT = np.ascontiguousarray(x.reshape(BS, D).T).astype(bf)  # [D, BS]
    mb = (np.asarray(attention_mask).astype(np.float32) - 1.0) * np.float32(1e9)
    mb = np.ascontiguousarray(
        mb.reshape(B, NB, P).transpose(2, 0, 1).reshape(P, B * NB))
    # causal boundary triangle (additive): 0 where col >= row else -1e9;
    # reused for every diagonal 128-step.
    pp = np.arange(P)[:, None]
    cc = np.arange(P)[None, :]
    cm = np.where(cc >= pp, 0.0, -1e9).astype(np.float32)

    Wq = np.asarray(Wq, np.float32) * scale
    bq = np.asarray(bq, np.float32) * scale
    Wk = np.asarray(Wk, np.float32)
    bk = np.asarray(bk, np.float32)
    Wv = np.asarray(Wv, np.float32)
    Wp = np.asarray(Wp, np.float32)

    def wlayout(w):  # [D, P] -> [P, KD*P] (partition-major, contiguous DMA)
        return np.ascontiguousarray(
            w.reshape(KD, P, P).transpose(1, 0, 2).reshape(P, KD * P)
        ).astype(bf)

    in_maps = []
    for c in range(NCORES):
        cs = slice(c * P, (c + 1) * P)
        in_maps.append({
            "xT": xT,
            "wq": wlayout(Wq[:, cs]),
            "wk": wlayout(Wk[:, cs]),
            "wv": wlayout(Wv[:, cs]),
            "bq": np.ascontiguousarray(bq[cs].reshape(P, 1)),
            "bk": np.ascontiguousarray(bk[cs].reshape(P, 1)),
            "wp": np.ascontiguousarray(Wp[cs, :]).astype(bf),
            "maskb": mb,
            "cmask": cm,
        })
    return in_maps


def run(inputs, trace=False, tmpdir=None):
    """Compile (cached) + run on 8 cores. Returns (output, BassKernelResults)."""
    from concourse import bass_utils
    nc = _get_nc()
    in_maps = make_in_maps(**inputs)
    kwargs = {}
    if trace:
        kwargs = dict(trace=True, tmpdir=tmpdir)
    res = bass_utils.run_bass_kernel_spmd(
        nc, in_maps, core_ids=list(range(NCORES)), **kwargs)
    acc = np.zeros((BS, D), dtype=np.float64)
    for r in res.results:
        acc += r["yp"].astype(np.float64)
    # V-bias commutes through softmax (weights sum to 1): + bv @ Wp
    bias = (np.asarray(inputs["bp"], np.float64)
            + np.asarray(inputs["bv"], np.float64)
            @ np.asarray(inputs["Wp"], np.float64))
    out = (acc + bias[None, :]).astype(np.float32)
    return out.reshape(B, S, D), res


def kernel(**inputs) -> np.ndarray:
    out, _ = run(inputs, trace=False)
    return out
